# revision 1
# baseline (speedup 1.0000x reference)
"""Expert-parallel MoE conditional feed-forward for 8 Trainium2 NeuronCores.

Problem: x[16,1024], expert_indices[16,2], gate/down_proj[8,2816,1024],
up_proj[8,1024,2816]. Reference computes, per (token, slot) pair with
e = expert_indices[t, a]:
    out[t,a,:] = (silu(x @ gate_proj[e].T) * (x @ down_proj[e].T)) @ up_proj[e].T

Sharding: core k owns expert k and computes its FFN output for ALL 16
tokens (the compute is negligible; the kernel is weight-streaming bound).
The host then gathers rows per expert_indices. This needs no indices on
device and is load-balanced regardless of routing.

Device kernel (per core): loop over 22 chunks of the 2816-wide
intermediate dim. Weights are host-packed into W[22, 128, 3072] so each
chunk is one contiguous 1.5 MB DMA:
    W[oc,p, 0:1024]     gate lhsT blocks  [h-frag p, o-frag m] per h-chunk
    W[oc,p, 1024:2048]  down lhsT blocks
    W[oc,p, 2048:3072]  up   rhs block    [o-frag p, j]
Per chunk: 8+8 accumulated matmuls -> psum_g/psum_d [128,16] (o on
partitions, tokens free), silu on ACT, mul on DVE, then 2 matmuls
(N=512) accumulating into a persistent psum_out[16,1024].
"""

import sys

for _p in ("/opt/trn_rl_repo", "/opt/pypackages"):
    if _p not in sys.path:
        sys.path.append(_p)

import numpy as np

NUM_EXPERTS = 8
HIDDEN = 1024
INTER = 2816
T = 16
N_CORES = 8
P = 128
OC = INTER // P   # 22 intermediate chunks
HC = HIDDEN // P  # 8 hidden chunks
JB = HIDDEN // 512  # 2 psum banks for the output accumulator

_COMPILED = None
LAST_RESULTS = None
TRACE = False


def _build():
    import concourse.bacc as bacc
    import concourse.bass as bass
    import concourse.tile as tile
    from concourse import mybir

    f32 = mybir.dt.float32
    nc = bacc.Bacc("TRN2", target_bir_lowering=False, debug=False,
                   num_devices=N_CORES)
    xt_d = nc.dram_tensor("xt", [P, HC * T], f32, kind="ExternalInput")
    w_d = nc.dram_tensor("w", [OC, P, 3 * HIDDEN], f32, kind="ExternalInput")
    out_d = nc.dram_tensor("out", [T, HIDDEN], f32, kind="ExternalOutput")

    with tile.TileContext(nc) as tc:
        with (
            tc.tile_pool(name="xp", bufs=1) as xp,
            tc.tile_pool(name="wp", bufs=4) as wp,
            tc.tile_pool(name="ip", bufs=4) as ip,
            tc.tile_pool(name="pg", bufs=2, space=bass.MemorySpace.PSUM) as pgp,
            tc.tile_pool(name="po", bufs=1, space=bass.MemorySpace.PSUM) as pop,
            tc.tile_pool(name="op", bufs=1) as op,
        ):
            xt = xp.tile([P, HC * T], f32)
            nc.sync.dma_start(xt[:], xt_d.ap())

            psum_out = pop.tile([T, HIDDEN], f32)

            for oc in range(OC):
                w = wp.tile([P, 3 * HIDDEN], f32)
                nc.sync.dma_start(w[:], w_d.ap()[oc])

                pg = pgp.tile([P, T], f32)
                pd = pgp.tile([P, T], f32)
                for hc in range(HC):
                    nc.tensor.matmul(
                        pg[:], w[:, hc * P:(hc + 1) * P], xt[:, hc * T:(hc + 1) * T],
                        start=(hc == 0), stop=(hc == HC - 1),
                    )
                for hc in range(HC):
                    nc.tensor.matmul(
                        pd[:], w[:, HIDDEN + hc * P:HIDDEN + (hc + 1) * P],
                        xt[:, hc * T:(hc + 1) * T],
                        start=(hc == 0), stop=(hc == HC - 1),
                    )

                s1 = ip.tile([P, T], f32)
                nc.scalar.activation(s1[:], pg[:],
                                     mybir.ActivationFunctionType.Silu)
                inter = ip.tile([P, T], f32)
                nc.vector.tensor_mul(inter[:], s1[:], pd[:])

                for jb in range(JB):
                    nc.tensor.matmul(
                        psum_out[:, jb * 512:(jb + 1) * 512], inter[:],
                        w[:, 2 * HIDDEN + jb * 512:2 * HIDDEN + (jb + 1) * 512],
                        start=(oc == 0), stop=(oc == OC - 1),
                    )

            out_sb = op.tile([T, HIDDEN], f32)
            nc.vector.tensor_copy(out_sb[:], psum_out[:])
            nc.sync.dma_start(out_d.ap(), out_sb[:])

    nc.compile()
    return nc


def _get_compiled():
    global _COMPILED
    if _COMPILED is None:
        _COMPILED = _build()
    return _COMPILED


def _pack_inputs(x, gate_proj, up_proj, down_proj):
    x = np.ascontiguousarray(x, dtype=np.float32)
    # xt[p, hc*T + t] = x[t, hc*128 + p]
    xt = np.ascontiguousarray(
        x.T.reshape(HC, P, T).transpose(1, 0, 2).reshape(P, HC * T))
    in_maps = []
    for k in range(N_CORES):
        g = np.asarray(gate_proj[k], dtype=np.float32)
        d = np.asarray(down_proj[k], dtype=np.float32)
        u = np.asarray(up_proj[k], dtype=np.float32)
        # Wg[oc, p, hc*128 + m] = g[oc*128 + m, hc*128 + p]
        wg = g.reshape(OC, P, HC, P).transpose(0, 3, 2, 1).reshape(OC, P, HIDDEN)
        wd = d.reshape(OC, P, HC, P).transpose(0, 3, 2, 1).reshape(OC, P, HIDDEN)
        # Wu[oc, p, j] = u[j, oc*128 + p]
        wu = u.reshape(HIDDEN, OC, P).transpose(1, 2, 0)
        w = np.ascontiguousarray(
            np.concatenate([wg, wd, wu], axis=2), dtype=np.float32)
        in_maps.append({"xt": xt, "w": w})
    return in_maps


def kernel(x, expert_indices, gate_proj, up_proj, down_proj):
    global LAST_RESULTS
    from concourse.bass_utils import run_bass_kernel_spmd

    nc = _get_compiled()
    in_maps = _pack_inputs(x, gate_proj, up_proj, down_proj)
    res = run_bass_kernel_spmd(nc, in_maps, core_ids=list(range(N_CORES)),
                               trace=TRACE)
    LAST_RESULTS = res

    expert_outs = np.stack([res.results[k]["out"] for k in range(N_CORES)])
    idx = np.asarray(expert_indices).astype(np.int64)  # [T, TOP_K]
    return expert_outs[idx, np.arange(T)[:, None], :].astype(np.float32)


# revision 2
# speedup vs baseline: 1.5439x; 1.5439x over previous
"""Expert-parallel MoE conditional feed-forward for 8 Trainium2 NeuronCores.

Problem: x[16,1024], expert_indices[16,2], gate/down_proj[8,2816,1024],
up_proj[8,1024,2816]. Reference computes, per (token, slot) pair with
e = expert_indices[t, a]:
    out[t,a,:] = (silu(x @ gate_proj[e].T) * (x @ down_proj[e].T)) @ up_proj[e].T

Sharding: core k owns expert k and computes its FFN output for ALL 16
tokens (the compute is negligible; the kernel is weight-streaming bound).
The host then gathers rows per expert_indices. This needs no indices on
device and is load-balanced regardless of routing.

Device kernel (per core): loop over 11 chunks of 256 of the 2816-wide
intermediate dim. Weights are host-packed into W[11, 128, 6144] so each
chunk is one contiguous 3 MB DMA:
    W[c,p, 0:2048]      gate blocks:  [c,p, hc*256+o] = g[c*256+o, hc*128+p]
    W[c,p, 2048:4096]   down blocks:  same layout
    W[c,p, 4096:6144]   up   blocks:  [c,p, f*1024+j] = u[j, c*256+f*128+p]
All big matmuls stream the WEIGHT as the moving operand (the stationary
is a 16-column token tile), so there are no 128-column fp32 LDWEIGHTS.
Per chunk: 8+8 accumulated matmuls -> psum_g/psum_d [16,256], silu on
ACT, mul on DVE, PE-transpose of the [16,128] intermediate halves via an
identity matmul, then 4 matmuls (N=512) accumulating into a persistent
psum_out[16,1024].
"""

import sys

for _p in ("/opt/trn_rl_repo", "/opt/pypackages"):
    if _p not in sys.path:
        sys.path.append(_p)

import numpy as np

NUM_EXPERTS = 8
HIDDEN = 1024
INTER = 2816
T = 16
N_CORES = 8
P = 128
CW = 256                  # intermediate chunk width
NCHUNK = INTER // CW      # 11
HC = HIDDEN // P          # 8 hidden chunks
GD_OFF = HC * CW          # 2048: offset of down blocks in packed W
U_OFF = 2 * HC * CW       # 4096: offset of up blocks
WCOLS = U_OFF + 2 * HIDDEN  # 6144

_COMPILED = None
LAST_RESULTS = None
TRACE = False


def _build():
    import concourse.bacc as bacc
    import concourse.bass as bass
    import concourse.tile as tile
    from concourse import mybir

    f32 = mybir.dt.float32
    nc = bacc.Bacc("TRN2", target_bir_lowering=False, debug=False,
                   num_devices=N_CORES)
    xt_d = nc.dram_tensor("xt", [P, HC * T], f32, kind="ExternalInput")
    eye_d = nc.dram_tensor("eye", [T, T], f32, kind="ExternalInput")
    w_d = nc.dram_tensor("w", [NCHUNK, P, WCOLS], f32, kind="ExternalInput")
    out_d = nc.dram_tensor("out", [T, HIDDEN], f32, kind="ExternalOutput")

    with tile.TileContext(nc) as tc:
        with (
            tc.tile_pool(name="xp", bufs=1) as xp,
            tc.tile_pool(name="wp", bufs=3) as wp,
            tc.tile_pool(name="ip", bufs=4) as ip,
            tc.tile_pool(name="pg", bufs=2, space=bass.MemorySpace.PSUM) as pgp,
            tc.tile_pool(name="tp", bufs=2, space=bass.MemorySpace.PSUM) as tpp,
            tc.tile_pool(name="po", bufs=1, space=bass.MemorySpace.PSUM) as pop,
            tc.tile_pool(name="op", bufs=1) as op,
        ):
            xt = xp.tile([P, HC * T], f32)
            nc.sync.dma_start(xt[:], xt_d.ap())
            eye = xp.tile([T, T], f32)
            nc.sync.dma_start(eye[:], eye_d.ap())

            psum_out = pop.tile([T, HIDDEN], f32)

            for c in range(NCHUNK):
                w = wp.tile([P, WCOLS], f32)
                nc.sync.dma_start(w[:], w_d.ap()[c])

                pg = pgp.tile([T, CW], f32)
                pd = pgp.tile([T, CW], f32)
                for hc in range(HC):
                    nc.tensor.matmul(
                        pg[:], xt[:, hc * T:(hc + 1) * T],
                        w[:, hc * CW:(hc + 1) * CW],
                        start=(hc == 0), stop=(hc == HC - 1),
                    )
                for hc in range(HC):
                    nc.tensor.matmul(
                        pd[:], xt[:, hc * T:(hc + 1) * T],
                        w[:, GD_OFF + hc * CW:GD_OFF + (hc + 1) * CW],
                        start=(hc == 0), stop=(hc == HC - 1),
                    )

                s1 = ip.tile([T, CW], f32)
                nc.scalar.activation(s1[:], pg[:],
                                     mybir.ActivationFunctionType.Silu)
                inter = ip.tile([T, CW], f32)
                nc.vector.tensor_mul(inter[:], s1[:], pd[:])

                for f in range(CW // P):
                    tp = tpp.tile([P, T], f32)
                    # PE transpose: tp = inter[:, f].T via identity matmul
                    nc.tensor.matmul(tp[:], inter[:, f * P:(f + 1) * P], eye[:])
                    it = ip.tile([P, T], f32)
                    nc.vector.tensor_copy(it[:], tp[:])
                    for jb in range(HIDDEN // 512):
                        nc.tensor.matmul(
                            psum_out[:, jb * 512:(jb + 1) * 512], it[:],
                            w[:, U_OFF + f * HIDDEN + jb * 512:
                               U_OFF + f * HIDDEN + (jb + 1) * 512],
                            start=(c == 0 and f == 0),
                            stop=(c == NCHUNK - 1 and f == CW // P - 1),
                        )

            out_sb = op.tile([T, HIDDEN], f32)
            nc.vector.tensor_copy(out_sb[:], psum_out[:])
            nc.sync.dma_start(out_d.ap(), out_sb[:])

    nc.compile()
    return nc


def _get_compiled():
    global _COMPILED
    if _COMPILED is None:
        _COMPILED = _build()
    return _COMPILED


def _pack_inputs(x, gate_proj, up_proj, down_proj):
    x = np.ascontiguousarray(x, dtype=np.float32)
    # xt[p, hc*T + t] = x[t, hc*128 + p]
    xt = np.ascontiguousarray(
        x.T.reshape(HC, P, T).transpose(1, 0, 2).reshape(P, HC * T))
    eye = np.eye(T, dtype=np.float32)
    in_maps = []
    for k in range(N_CORES):
        g = np.asarray(gate_proj[k], dtype=np.float32)
        d = np.asarray(down_proj[k], dtype=np.float32)
        u = np.asarray(up_proj[k], dtype=np.float32)
        # Wg[c, p, hc*CW + o] = g[c*CW + o, hc*128 + p]
        wg = g.reshape(NCHUNK, CW, HC, P).transpose(0, 3, 2, 1).reshape(
            NCHUNK, P, HC * CW)
        wd = d.reshape(NCHUNK, CW, HC, P).transpose(0, 3, 2, 1).reshape(
            NCHUNK, P, HC * CW)
        # Wu[c, p, f*HIDDEN + j] = u[j, c*CW + f*128 + p]
        wu = u.reshape(HIDDEN, NCHUNK, CW // P, P).transpose(1, 3, 2, 0).reshape(
            NCHUNK, P, 2 * HIDDEN)
        w = np.ascontiguousarray(
            np.concatenate([wg, wd, wu], axis=2), dtype=np.float32)
        in_maps.append({"xt": xt, "eye": eye, "w": w})
    return in_maps


def kernel(x, expert_indices, gate_proj, up_proj, down_proj):
    global LAST_RESULTS
    from concourse.bass_utils import run_bass_kernel_spmd

    nc = _get_compiled()
    in_maps = _pack_inputs(x, gate_proj, up_proj, down_proj)
    res = run_bass_kernel_spmd(nc, in_maps, core_ids=list(range(N_CORES)),
                               trace=TRACE)
    LAST_RESULTS = res

    expert_outs = np.stack([res.results[k]["out"] for k in range(N_CORES)])
    idx = np.asarray(expert_indices).astype(np.int64)  # [T, TOP_K]
    return expert_outs[idx, np.arange(T)[:, None], :].astype(np.float32)
